# revision 7
# baseline (speedup 1.0000x reference)
"""Trainium2 Bass kernel for the EnforcedNeuralODE recurrence.

Reference (per timestep): x_t = Wx x_{t-1} + Wf f_{t-1} + b over T-1=4095
steps, batch 256, state 64, force 64.  Output [T, B, 64].

Algorithm (per core, 32-sample batch shard, all math bf16 / f32 PSUM):
  Bias fold: f'_t = f_t + Wf^{-1} b, so x_t = Wx x_{t-1} + Wf f'_t.
  Blocks of KB=32 steps; NB=128 blocks; chunk = 16 blocks (free dim
  N=512 cols = 16 blocks x 32 batch); 8 chunks in 2 groups of 4.
  Phase1  g31_blk = sum_j Wx^{31-j} Wf f'_j   (block forcing response;
          16 f-pair matmuls per chunk, accumulated in PSUM)
  P2      block start states s_b: superblock (8 blocks) convolution v_S
          (8 matmuls) + 16-hop scan  s_{8S+8} = Wx^256 s_{8S} + v_S
          + 7-step parallel reconstruction of interior entries.
  Phase2  x-chain per block pair-by-pair, two matmuls per pair tile
          [x_odd; x_even] (M=128), chained through the bf16 out staging:
            x_{2p+1} = Wx^2 x_{2p-1} + WxWf f'_{2p} + Wf f'_{2p+1}
            x_{2p}   = Wx   x_{2p-1} + Wf   f'_{2p}
  All matmuls K=128/M=128 (zero-padded lhsT), N=512 free, bf16 operands:
  uniform tile mode, one PE cost = N cycles; LDWEIGHTS overlaps.
  Groups pipeline: f DMA (sync+scalar HWDGE queues, 2MB/16KB-desc) ->
  phase1 -> P2 -> phase2 (group 1 phase1 interleaved into group 0
  phase2 to keep PE dense) -> out DMA (gpsimd SWDGE + sync queues).
"""

import numpy as np
from contextlib import ExitStack

NCORES = 8
BATCH, STATE, FDIM, TIMESPAN = 256, 64, 64, 4096

BC = BATCH // NCORES    # 32 batch per core
KB = 32                 # steps per block
PAIRS = KB // 2         # 16 step-pairs per block
NB = TIMESPAN // KB     # 128 blocks (4095 steps padded to 4096)
NBC = 16                # blocks per chunk
CHUNKS = NB // NBC      # 8
N = NBC * BC            # 512 free cols per (chunk, pair)
NG = 2                  # chunk groups (pipeline halves)
GC = CHUNKS // NG       # 4 chunks per group
SBK = 8                 # blocks per superblock (P2)
NSB = NB // SBK         # 16 superblocks
SBH = NSB // NG         # 8 superblocks per group

F_COLS = NG * PAIRS * GC * N      # 65536 forcing cols (bf16)
O_COLS = NG * PAIRS * GC * N      # 65536 output cols (bf16)
W_COLS = (PAIRS + 2 + 8 + 1) * 128       # 3456 weight cols
F_PIECES = [1, 1, 2, 4, 4, 4]     # f DMA sizes in pairs (graded arrival)

_NC_CACHE: dict = {}


def _build_nc():
    import concourse.bass as bass  # noqa: F401
    import concourse.tile as tile
    from concourse import bacc, mybir

    f32 = mybir.dt.float32
    bf16 = mybir.dt.bfloat16
    AF = mybir.ActivationFunctionType

    nc = bacc.Bacc("TRN2", target_bir_lowering=False, debug=False)

    f_dram = nc.dram_tensor("f", [128, F_COLS], bf16, kind="ExternalInput")
    w_dram = nc.dram_tensor("wts", [128, W_COLS], bf16, kind="ExternalInput")
    s0_dram = nc.dram_tensor("s0", [128, BC], bf16, kind="ExternalInput")
    out_dram = nc.dram_tensor("out", [128, O_COLS], bf16, kind="ExternalOutput")

    with tile.TileContext(nc) as tc, ExitStack() as ctx:
        singles = ctx.enter_context(tc.tile_pool(name="singles", bufs=1))
        opool = ctx.enter_context(tc.tile_pool(name="opool", bufs=4))
        psA = ctx.enter_context(tc.tile_pool(name="psA", bufs=4, space="PSUM"))
        psB = ctx.enter_context(tc.tile_pool(name="psB", bufs=4, space="PSUM"))

        fsb = singles.tile([128, F_COLS], bf16)
        wsb = singles.tile([128, W_COLS], bf16)
        s_sb = singles.tile([128, (NB + 1) * BC], bf16)
        g31 = singles.tile([128, NB * BC], bf16)
        vsb = singles.tile([128, NSB * BC], bf16)

        # weight slices
        def L1(p):
            return wsb[:, p * 128 : (p + 1) * 128]

        Lhx = wsb[:, 2048:2176]
        Lf = wsb[:, 2176:2304]

        def Lj(j):
            return wsb[:, 2304 + j * 128 : 2304 + (j + 1) * 128]

        Lrec = Lj(6)          # (Wx^32)^T
        Lscan = wsb[:, 3328:3456]

        def fv(g, p, ci):
            base = ((g * PAIRS + p) * GC + ci) * N
            return fsb[:, base : base + N]

        # ---- input DMAs: graded f pieces round-robin over 3 queues ----
        nc.scalar.dma_start(out=wsb[:, 0:2048], in_=w_dram[:, 0:2048])   # L1 wts
        nc.gpsimd.dma_start(out=wsb[:, 2048:], in_=w_dram[:, 2048:])
        nc.sync.dma_start(out=s_sb[:, 0:BC], in_=s0_dram[:])
        fq = [nc.sync, nc.scalar, nc.gpsimd]
        for g in range(NG):
            p0 = 0
            for i, sz in enumerate(F_PIECES):
                c0 = (g * PAIRS + p0) * GC * N
                c1 = (g * PAIRS + p0 + sz) * GC * N
                fq[i % 3].dma_start(out=fsb[:, c0:c1], in_=f_dram[:, c0:c1])
                p0 += sz

        # strided views for P2: [128, 8 superblocks, 32] at offset j
        def g31v(g, j):
            r = g31[:, g * 2048 : (g + 1) * 2048].rearrange(
                "p (s j b) -> p s j b", s=SBH, j=SBK, b=BC
            )
            return r[:, :, j, :]

        def sv(g, k):
            r = s_sb[:, g * 2048 : (g + 1) * 2048].rearrange(
                "p (s j b) -> p s j b", s=SBH, j=SBK, b=BC
            )
            return r[:, :, k, :]

        def phase1_evac(g, ci, acc, eng):
            c = g * GC + ci
            eng_map = {0: nc.scalar, 1: nc.vector}
            e = eng_map[eng]
            if e is nc.scalar:
                e.activation(g31[:, c * N : (c + 1) * N], acc[:], AF.Copy)
            else:
                e.tensor_copy(g31[:, c * N : (c + 1) * N], acc[:])

        # ---- phase1 group 0 (sweep-major: follows f DMA arrival) ----
        accs0 = [psA.tile([128, N], f32, tag="A", name=f"acc0_{ci}") for ci in range(GC)]
        for p in range(PAIRS):
            for ci in range(GC):
                nc.tensor.matmul(
                    accs0[ci][:], L1(p), fv(0, p, ci),
                    start=(p == 0), stop=(p == PAIRS - 1),
                )
        for ci in range(GC):
            phase1_evac(0, ci, accs0[ci], ci % 2)

        # ---- P2 for one group ----
        def p2(g):
            vt = psA.tile([128, N], f32, tag="A", name="vt")
            for j in range(SBK):
                nc.tensor.matmul(
                    vt[:, 0 : SBH * BC], Lj(j), g31v(g, j),
                    start=(j == 0), stop=(j == SBK - 1),
                )
            nc.scalar.activation(
                vsb[:, g * SBH * BC : (g + 1) * SBH * BC], vt[:, 0 : SBH * BC], AF.Copy
            )
            for s in range(SBH):
                S = g * SBH + s
                e_in, e_out = S * SBK, S * SBK + SBK
                pt = psA.tile([128, N], f32, tag="A", name="pt")
                nc.tensor.matmul(
                    pt[:, 0:BC], Lscan,
                    s_sb[:, e_in * BC : (e_in + 1) * BC],
                    start=True, stop=True,
                )
                nc.vector.tensor_add(
                    s_sb[:, e_out * BC : (e_out + 1) * BC],
                    pt[:, 0:BC],
                    vsb[:, S * BC : (S + 1) * BC],
                )
            for k in range(1, SBK):
                rt = psA.tile([128, N], f32, tag="A", name="rt")
                nc.tensor.matmul(
                    rt[:, 0 : SBH * BC], Lrec, sv(g, k - 1), start=True, stop=True
                )
                nc.vector.tensor_add(sv(g, k), rt[:, 0 : SBH * BC], g31v(g, k - 1))

        p2(0)

        # ---- phase2 for group g; during g=0 interleave group-1 phase1 ----
        def phase2(g, interleave):
            il_accs = {}
            ost_prev = None
            for p in range(PAIRS):
                ost = opool.tile([128, GC * N], bf16, tag="ost", name="ost")
                chain = [psB.tile([128, N], f32, tag="B", name=f"ch{ci}") for ci in range(GC)]
                for ci in range(GC):
                    if p == 0:
                        c = g * GC + ci
                        prev = s_sb[:, c * NBC * BC : (c + 1) * NBC * BC]
                    else:
                        prev = ost_prev[:, ci * N : (ci + 1) * N]
                    nc.tensor.matmul(chain[ci][:], Lhx, prev, start=True, stop=False)
                for ci in range(GC):
                    nc.tensor.matmul(
                        chain[ci][:], Lf, fv(g, p, ci), start=False, stop=True
                    )
                if interleave:
                    # group-1 phase1, chunk-pair-major: sweeps 0-7 chunks
                    # (0,1), sweeps 8-15 chunks (2,3); 2 pairs per sweep
                    cpair = p // 8
                    q0 = (p % 8) * 2
                    for dci in range(2):
                        ci1 = cpair * 2 + dci
                        if q0 == 0:
                            il_accs[ci1] = psA.tile(
                                [128, N], f32, tag="A", name=f"il{ci1}"
                            )
                        for q in (q0, q0 + 1):
                            nc.tensor.matmul(
                                il_accs[ci1][:], L1(q), fv(1, q, ci1),
                                start=(q == 0), stop=(q == PAIRS - 1),
                            )
                    if p % 8 == 7:
                        for dci in range(2):
                            ci1 = cpair * 2 + dci
                            phase1_evac(1, ci1, il_accs[ci1], dci)
                for ci in range(GC):
                    dst = ost[:, ci * N : (ci + 1) * N]
                    if (p * GC + ci) % 2 == 0:
                        nc.scalar.activation(dst, chain[ci][:], AF.Copy)
                    else:
                        nc.vector.tensor_copy(dst, chain[ci][:])
                base = (g * PAIRS + p) * GC * N
                oq = [nc.gpsimd, nc.sync, nc.scalar]
                oq[(g * PAIRS + p) % 3].dma_start(
                    out=out_dram[:, base : base + GC * N], in_=ost[:]
                )
                ost_prev = ost

        phase2(0, interleave=True)
        p2(1)
        phase2(1, interleave=False)

    nc.compile()
    return nc


def _get_nc():
    if "nc" not in _NC_CACHE:
        _NC_CACHE["nc"] = _build_nc()
    return _NC_CACHE["nc"]


def _host_prep(inputs, forcing, fc_w, fc_b):
    """Build per-core input maps (numpy only, untimed)."""
    import ml_dtypes

    bf = ml_dtypes.bfloat16
    inputs = np.asarray(inputs, np.float32)
    fc_w = np.asarray(fc_w, np.float32)
    fc_b = np.asarray(fc_b, np.float32)
    Wx = fc_w[:, :STATE].astype(np.float64)
    Wf = fc_w[:, STATE:].astype(np.float64)
    b = fc_b.astype(np.float64)
    c = np.linalg.solve(Wf, b)

    # powers of Wx
    WxP = {}
    P = np.eye(STATE)
    for j in range(33):
        WxP[j] = P
        P = Wx @ P
    W256 = np.linalg.matrix_power(Wx, 256)

    wts = np.zeros((128, W_COLS), np.float32)
    for p in range(PAIRS):
        wts[0:64, p * 128 : p * 128 + 64] = (WxP[31 - 2 * p] @ Wf).T
        wts[64:128, p * 128 : p * 128 + 64] = (WxP[30 - 2 * p] @ Wf).T
    wts[0:64, 2048:2112] = (WxP[2]).T          # Lhx: x_odd <- Wx^2 x
    wts[0:64, 2112:2176] = Wx.T                # Lhx: x_even <- Wx x
    wts[0:64, 2176:2240] = (Wx @ Wf).T         # Lf: x_odd <- WxWf f0
    wts[0:64, 2240:2304] = Wf.T                # Lf: x_even <- Wf f0
    wts[64:128, 2176:2240] = Wf.T              # Lf: x_odd <- Wf f1
    for j in range(8):
        wts[0:64, 2304 + j * 128 : 2304 + j * 128 + 64] = (
            np.linalg.matrix_power(Wx, (7 - j) * KB)
        ).T
    wts[0:64, 3328:3392] = W256.T
    wts = wts.astype(bf)

    # forcing with bias fold, padded to T steps
    fp = np.zeros((TIMESPAN, BATCH, FDIM), np.float32)
    fp[: TIMESPAN - 1] = np.asarray(forcing, np.float32) + c.astype(np.float32)
    fp[TIMESPAN - 1] = c.astype(np.float32)
    # [Bk, pair, parity, batch, feat] -> [g, ci, blk, p, par, b, feat]
    arr = fp.reshape(NG, GC, NBC, PAIRS, 2, BATCH, FDIM)
    # dram[par*64+feat, ((g*16+p)*4+ci)*512 + blk*32 + b]
    arr = arr.transpose(4, 6, 0, 3, 1, 2, 5)  # [par, feat, g, p, ci, blk, b]

    in_maps = []
    for core in range(NCORES):
        bs = slice(core * BC, (core + 1) * BC)
        fcore = np.ascontiguousarray(arr[..., bs]).reshape(128, F_COLS).astype(bf)
        s0 = np.zeros((128, BC), np.float32)
        s0[0:64] = inputs[bs].T
        in_maps.append({"f": fcore, "wts": wts, "s0": s0.astype(bf)})
    return in_maps


def _host_decode(results, inputs):
    """Per-core out [128, O_COLS] bf16 -> full [T, B, S] f32."""
    inputs = np.asarray(inputs, np.float32)
    out = np.empty((TIMESPAN, BATCH, STATE), np.float32)
    out[0] = inputs
    for core in range(NCORES):
        o = np.asarray(results[core]["out"], dtype=np.float32)
        # [par, feat, g, p, ci, blk, b]; par 0 = x_{2p+1}, 1 = x_{2p}
        o = o.reshape(2, 64, NG, PAIRS, GC, NBC, BC)
        # -> [g, ci, blk, p, parflip(even first), b, feat]
        o = o[::-1].transpose(2, 4, 5, 3, 0, 6, 1)
        o = o.reshape(TIMESPAN, BC, STATE)
        out[1:, core * BC : (core + 1) * BC] = o[: TIMESPAN - 1]
    return out


def kernel(inputs, forcing, fc_w, fc_b, timespan):
    from concourse.bass_utils import run_bass_kernel_spmd

    timespan = int(timespan)
    assert timespan == TIMESPAN, f"hardcoded for timespan={TIMESPAN}, got {timespan}"
    nc = _get_nc()
    in_maps = _host_prep(inputs, forcing, fc_w, fc_b)
    res = run_bass_kernel_spmd(nc, in_maps, core_ids=list(range(NCORES)))
    return _host_decode(res.results, inputs)


if __name__ == "__main__":
    nc = _get_nc()
    print("built ok")
